# revision 70
# baseline (speedup 1.0000x reference)
"""CenterlineLoss Trainium2 kernel.

Computes 0.5*(mean1 + mean2) where
  mean1 = mean over valid proj points of distance to nearest ref point
  mean2 = mean over ref points of distance to nearest valid proj point
(reference semantics: ref coords swapped (y,x); proj row-reversal is a
permutation and does not affect either reduction; proj validity mask
applied to both reductions).

Strategy: the host drops out-of-image proj rows entirely (compaction) and
pads the survivors to a multiple of 128*NCORES, so the device computes a
[1792, 8192] squared-distance block per core instead of [2048, 8192]
(~12.5% less work; the pads duplicate valid rows, harmless for the
column mins and ignored for the row means).  TensorE produces d^2 via a
K=14 fp16 limb-split matmul (exact encoding, fp32 PSUM accumulate), in
14 tiles of [128, 8192] per core, 4 PSUM rounds of 2048 cols each.

Steady state is ScalarE-evacuation-bound with every engine load-assigned:
  ScalarE  copies PSUM -> fp16 SBUF (~7.6us/tile, the bottleneck),
  VectorE  runs the running column-min (colacc), the level-1 row fold
           (pairing rounds 0<->2, 1<->3) and half a level-2 fold
           (~7.4us/tile),
  TensorE  16 matmuls/tile (~3.4us, p-state warm),
  DMA      streams out [128, 3072] row-min partials per tile.
(GPSIMD/Pool cannot execute TensorTensor on this toolchain, so it only
carries input-DMA descriptor generation.)

The last three tiles ship their raw fp16 distance rounds instead of
reducing on-device, so no reduction or colacc-output work hangs off the
pipeline's end: the colacc (final after tile 10) and the raw rounds
stream out overlapped with those tiles' compute, whose evacuation load
is itself split ScalarE/VectorE (rounds 0,1 / 2,3).

The host finishes the tiny reductions (min over shipped partials / min
over partitions+cores), the masked means and the sqrt in fp64.
"""

import time

import numpy as np

import concourse.bacc as bacc
import concourse.mybir as mybir
import concourse.tile as tile
from concourse import bass_utils

N = 16384
M = 8192
NCORES = 8
K = 14                      # limb-split contraction depth
P2SCALE = 64.0
R2SCALE = 16.0
CENTER = (320.0, 240.0)

# padded-row capacity: 14336 = 112 tiles of 128 (14 per core).  valid
# count for the uniform input distribution is ~13828 (11 sigma below
# 14336); anything larger falls back to a full-size program.
NPAD_DEFAULT = 14336

_f16 = np.float16


def _split2(v):
    h = v.astype(_f16).astype(np.float64)
    l = (v - h).astype(_f16).astype(np.float64)
    return h, l


def _split3(v):
    h = v.astype(_f16).astype(np.float64)
    r = v - h
    m = r.astype(_f16).astype(np.float64)
    l = (r - m).astype(_f16).astype(np.float64)
    return h, m, l


def _host_prep(proj_f32, ref_f32):
    proj = proj_f32.astype(np.float64)
    refs = ref_f32.astype(np.float64)[:, ::-1]  # torch flip(1): swap (x,y)

    mask = (
        (proj[:, 0] >= 0.0) & (proj[:, 0] <= 640.0)
        & (proj[:, 1] >= 0.0) & (proj[:, 1] <= 480.0)
    )
    valid = np.flatnonzero(mask)
    n_valid = len(valid)

    npad = NPAD_DEFAULT
    while npad < n_valid:        # fallback capacity (never for uniform fill)
        npad += 128 * NCORES
    idx = np.concatenate([valid, np.repeat(valid[:1], npad - n_valid)])

    pts = proj[idx]              # compacted + padded valid points

    c = np.array(CENTER)
    pt = pts - c
    rt = refs - c

    Xh, Xl = _split2(pt[:, 0])
    Yh, Yl = _split2(pt[:, 1])
    Xh_, Xl_ = _split2(rt[:, 0])
    Yh_, Yl_ = _split2(rt[:, 1])

    px, py = Xh + Xl, Yh + Yl          # the exactly-represented points
    rx, ry = Xh_ + Xl_, Yh_ + Yl_
    P2a, P2b, P2c = _split3((px * px + py * py) / P2SCALE)
    R2a, R2b, R2c = _split3((rx * rx + ry * ry) / R2SCALE)

    rs = np.full(npad, R2SCALE)
    a = np.stack([Xh, Xh, Xl, Xl, Yh, Yh, Yl, Yl, P2a, P2b, P2c, rs, rs, rs])
    ps = np.full(M, P2SCALE)
    b = np.stack([-2 * Xh_, -2 * Xl_, -2 * Xh_, -2 * Xl_,
                  -2 * Yh_, -2 * Yl_, -2 * Yh_, -2 * Yl_,
                  ps, ps, ps, R2a, R2b, R2c])

    return a.astype(_f16), b.astype(_f16), n_valid, npad


_PROGRAM_CACHE = {}


def _raw_tiles(ntiles):
    """Raw-shipped tiles: the final three, so the colacc output DMA and
    the raw-tile DMAs overlap the last three tile periods."""
    return (ntiles - 3, ntiles - 2, ntiles - 1)


def _build_program(ntiles=NPAD_DEFAULT // (128 * NCORES)):
    if ntiles in _PROGRAM_CACHE:
        return _PROGRAM_CACHE[ntiles]

    f16 = mybir.dt.float16
    f32 = mybir.dt.float32
    MIN = mybir.AluOpType.min
    nloc = ntiles * 128

    nc = bacc.Bacc("TRN2", target_bir_lowering=False, debug=False,
                   num_devices=NCORES)

    a_dram = nc.dram_tensor("a_in", [K, nloc], f16, kind="ExternalInput").ap()
    b_dram = nc.dram_tensor("b_in", [K, M], f16, kind="ExternalInput").ap()
    # most tiles emit their fold-level-1 row partials ([128, 4096]); the
    # final NRAW tiles ship their raw fp16 distance rounds instead (host
    # folds those into both reductions) so no reduction work hangs off
    # the pipeline's end and the output DMA spreads over several periods
    NRAW = 3
    raw_set = _raw_tiles(ntiles)
    rowp_dram = nc.dram_tensor("rowpart_out", [128, (ntiles - NRAW) * 3072],
                               f16, kind="ExternalOutput").ap()
    dtl_dram = nc.dram_tensor("dtlast_out", [128, NRAW * M], f16,
                              kind="ExternalOutput").ap()
    colm_dram = nc.dram_tensor("colacc_out", [128, M], f16,
                               kind="ExternalOutput").ap()

    with tile.TileContext(nc) as tc, \
            tc.tile_pool(name="const", bufs=1) as const_pool:
        a_sb = const_pool.tile([K, nloc], f16, tag="a_sb")
        b_sb = const_pool.tile([K, M], f16, tag="b_sb")
        colacc = const_pool.tile([128, M], f16, tag="colacc")
        warm = const_pool.tile([1, 8], f16, tag="warm")

        # trigger the ACT function-table load while DMAs are in flight
        nc.scalar.copy(warm[:, 4:], warm[:, :4])
        # spread the input DMAs across DGE paths so the first round's
        # operands land in parallel, not serialized on one queue; the
        # very first 512-col piece rides alone for the earliest start
        nc.gpsimd.dma_start(a_sb[:], a_dram)
        nc.sync.dma_start(b_sb[:, :512], b_dram[:, :512])
        nc.sync.dma_start(b_sb[:, 512:3072], b_dram[:, 512:3072])
        nc.gpsimd.dma_start(b_sb[:, 3072:5632], b_dram[:, 3072:5632])
        nc.gpsimd.dma_start(b_sb[:, 5632:], b_dram[:, 5632:])

        with (
            tc.tile_pool(name="mmpsum", bufs=2, space="PSUM") as psum_pool,
            tc.tile_pool(name="data", bufs=3) as data_pool,
            tc.tile_pool(name="s1p", bufs=2) as s1_pool,
        ):
            # per tile: 4 rounds of 2048 cols.  ScalarE evacuates (the
            # bottleneck); VectorE runs the running column-min and the
            # level-1 row fold, whose [128, 4096] output ships to the host
            # (GPSIMD cannot run TensorTensor on this toolchain, so the
            # deeper fold levels are finished host-side).
            n_raw_seen = 0
            n_red_seen = 0
            last_red = max(t for t in range(ntiles) if t not in raw_set)
            for t in range(ntiles):
                lhsT = a_sb[:, t * 128:(t + 1) * 128]
                raw = t in raw_set
                dts = []
                s1 = None
                for r in range(4):
                    cr = slice(r * 2048, (r + 1) * 2048)
                    ps_t = psum_pool.tile([128, 2048], f32, tag="mm")
                    # tile 0 round 0: evacuate in pieces so ScalarE starts
                    # after two matmuls (shorter pipeline-fill head)
                    parts = [(0, 2), (2, 4)] if t == 0 and r == 0 else [(0, 4)]
                    # raw tiles: VectorE (idle there) evacuates rounds 2,3
                    # -- with 2 PSUM buffers, round r+2's matmuls wait on
                    # round r's evacuation, so the slower VectorE copies
                    # must not gate rounds that follow soon
                    dve_evac = raw and r in (2, 3)
                    dt_r = data_pool.tile([128, 2048], f16, tag=f"dt{r}")
                    for (q0, q1) in parts:
                        for q in range(q0, q1):
                            cc = r * 4 + q
                            nc.tensor.matmul(
                                ps_t[:, q * 512:(q + 1) * 512],
                                lhsT,
                                b_sb[:, cc * 512:(cc + 1) * 512],
                                start=True, stop=True,
                            )
                        # fp32 PSUM -> fp16 SBUF -- the evacuation
                        if dve_evac:
                            nc.vector.tensor_copy(
                                dt_r[:, q0 * 512:q1 * 512],
                                ps_t[:, q0 * 512:q1 * 512])
                        else:
                            nc.scalar.copy(dt_r[:, q0 * 512:q1 * 512],
                                           ps_t[:, q0 * 512:q1 * 512])
                    dts.append(dt_r)
                    if raw:
                        # ship the raw round as it lands; the host folds it
                        # into both reductions
                        off = n_raw_seen * M
                        nc.sync.dma_start(
                            dtl_dram[:, off + r * 2048:off + (r + 1) * 2048],
                            dt_r[:])
                        continue
                    # column minima: running elementwise min per round slice
                    if n_red_seen == 0:
                        nc.vector.tensor_copy(colacc[:, cr], dt_r[:])
                    else:
                        nc.vector.tensor_tensor(colacc[:, cr], dt_r[:],
                                                colacc[:, cr], op=MIN)
                    if t == last_red:
                        # colacc slice r is final; overlap its DMA with the
                        # remaining pipeline
                        nc.sync.dma_start(colm_dram[:, cr], colacc[:, cr])
                    # row fold level 1: pair round 0<->2, 1<->3
                    if r >= 2:
                        if s1 is None:
                            s1 = s1_pool.tile([128, 4096], f16, tag="s1")
                        lo = (r - 2) * 2048
                        nc.vector.tensor_tensor(s1[:, lo:lo + 2048],
                                                dts[r - 2][:], dt_r[:],
                                                op=MIN)
                        if r == 3:
                            nc.sync.dma_start(
                                rowp_dram[:, n_red_seen * 3072 + 1024:
                                          n_red_seen * 3072 + 3072],
                                s1[:, 2048:])
                if raw:
                    n_raw_seen += 1
                    continue
                # half a level-2 fold (what fits VectorE's slack) shrinks
                # the shipped row partial from 4096 to 3072 columns
                nc.vector.tensor_tensor(s1[:, :1024], s1[:, :1024],
                                        s1[:, 1024:2048], op=MIN)
                nc.sync.dma_start(
                    rowp_dram[:, n_red_seen * 3072:n_red_seen * 3072 + 1024],
                    s1[:, :1024])
                n_red_seen += 1

    nc.compile()
    _PROGRAM_CACHE[ntiles] = nc
    return nc


def _run_on_hw(a, b, ntiles, trace=False, tmpdir=None):
    nc = _build_program(ntiles)
    nloc = ntiles * 128
    in_maps = [
        {
            "a_in": np.ascontiguousarray(a[:, c * nloc:(c + 1) * nloc]),
            "b_in": b,
        }
        for c in range(NCORES)
    ]
    # transient NRT_EXEC_UNIT_UNRECOVERABLE states clear after the worker
    # recycles; retry with increasing waits
    last = None
    for wait_s in (0, 30, 60, 90):
        if wait_s:
            time.sleep(wait_s)
        try:
            return bass_utils.run_bass_kernel_spmd(
                nc, in_maps, core_ids=list(range(NCORES)), trace=trace,
                tmpdir=tmpdir,
            )
        except Exception as e:
            last = e
    raise last


def kernel(bezier_proj_centerline_img, ref_catheter_centerline, _trace=False,
           _tmpdir=None):
    a, b, n_valid, npad = _host_prep(
        np.asarray(bezier_proj_centerline_img, dtype=np.float32),
        np.asarray(ref_catheter_centerline, dtype=np.float32),
    )
    ntiles = npad // (128 * NCORES)
    nloc = ntiles * 128

    res = _run_on_hw(a, b, ntiles, trace=_trace, tmpdir=_tmpdir)

    raw_set = _raw_tiles(ntiles)
    red_tiles = [t for t in range(ntiles) if t not in raw_set]
    rowmins = np.empty(npad, np.float64)
    colmin = np.full(M, np.inf)
    for c in range(NCORES):
        out = res.results[c]
        rp = out["rowpart_out"].astype(np.float64)   # [128, n_red*3072]
        dl = out["dtlast_out"].astype(np.float64)    # [128, n_raw*M]
        ca = out["colacc_out"].astype(np.float32)    # [128, M]
        base = c * nloc
        # rowpart col block k holds the 3072 fold survivors of the k-th
        # reduced tile; dtl block k is the k-th raw tile, shipped whole
        rp = rp.reshape(128, len(red_tiles), 3072).min(axis=2)
        for k, t in enumerate(red_tiles):
            rowmins[base + t * 128:base + (t + 1) * 128] = rp[:, k]
        dl = dl.reshape(128, len(raw_set), M)
        for k, t in enumerate(raw_set):
            rowmins[base + t * 128:base + (t + 1) * 128] = \
                dl[:, k, :].min(axis=1)
        colmin = np.minimum(colmin, ca.min(axis=0).astype(np.float64))
        colmin = np.minimum(colmin, dl.min(axis=(0, 1)))

    mean1 = np.sqrt(np.maximum(rowmins[:n_valid], 0.0)).mean()
    mean2 = np.sqrt(np.maximum(colmin, 0.0)).mean()
    out = np.float32(0.5 * (mean1 + mean2))
    if _trace:
        return out, res
    return out
